# revision 15
# baseline (speedup 1.0000x reference)
"""CE + CJS loss kernel for Trainium2, data-parallel over 8 NeuronCores.

Math (reference):
    logp = log_softmax(pred_logit, axis=1)          # x - lse_i
    ce   = -mean_i( sum_j gt*logp )
    p    = softmax(pred_logit)
    m    = 0.5*(gt + p + EPS)
    contrib = gt*ln(gt) + p*logp - (gt+p)*ln(m)     # per element
    cjs  = 0.5 * sum_ij w_j * contrib_ij / B,  w_j = C - j
    loss = ce + 0.5*cjs

v2 design (vs the first working kernel: 241us HW):
  - Inputs are downcast to bf16 on the host: halves DMA (32MiB->16MiB
    per core) and puts every elementwise op in a fast mode.
  - 4-product form: contrib = g*lng + p*xp - u*logm with u = g+p.
    Column sums are PE matmuls with a ones vector; the minus sign is a
    neg_ones stationary vector, so no tensor-level subtracts are needed.
    With the same +K shift on lng, logm, and xp the shift cancels
    exactly: K*(g + p - u) = 0.
  - Engine balance per 2048-col chunk:
      ACT: lng=Ln(EK*g), logm=Ln(EK/2*u + eps'), (+exp pass amortized)
      DVE: xp, p (tensor_scalar 4x), u, Pb, Pa, Pc (tensor_tensor 2x)
      GpSimd: Pd = g*xp (CE plane) -- keeps it off the two hot engines
      PE : ones/neg_ones column-sum matmuls into PSUM
  - CE total = sum(Pd colsums) - K*ROWS per core (rows of gt sum to 1).
"""
import os

import numpy as np
import ml_dtypes

import concourse.bass as bass
import concourse.tile as tile
from concourse import mybir
from concourse.bass_utils import run_bass_kernel_spmd
from concourse.vector_clock import ScopedClock

B, C = 4096, 8192
N_CORES = 8
ROWS = B // N_CORES          # 512 rows per core
N_BLK = ROWS // 128          # 4 partition blocks
F2 = 2048                    # chunk width
N_CHUNK = C // F2            # chunks per block
N_SLICE = C // 512           # matmul column slices
EPS = 1e-8
# All log-magnitude tensors (lng, logm, x-lse) sit near -9.2 where bf16's
# ulp is 0.0625; shifting them by +K before the bf16 round shrinks the
# quantization bias ~8x. The same K on lng/logm/xp cancels exactly in
# g*lng + p*xp - u*logm; the CE plane carries it and the host removes it.
K_SHIFT = 9.2
_EK = float(np.float32(np.exp(K_SHIFT)))          # scale for Ln(gt)
_EK_HALF = float(np.float32(_EK) / np.float32(2.0))  # exact /2: same shift
_EMK = float(np.float32(np.exp(-K_SHIFT)))        # scale for lse
K_LSE = -float(np.log(np.float64(_EMK)))          # effective shift on x-lse

PD_ON_GPSIMD = True   # CE plane g*xp on GpSimd instead of DVE

f32 = mybir.dt.float32
bf16 = mybir.dt.bfloat16
AF = mybir.ActivationFunctionType
ALU = mybir.AluOpType


def _patched_drain_and_barrier(self, tick_clock, wait_clock):
    # Walrus CoreV3 codegen allows only ONE sync-wait command on a
    # Drain/NoOp (NO_STRUCT ctrl). The stock Tile tail drain carries one
    # wait per pending engine clock and fails to compile. Split the waits
    # across single-wait SP nops; SP executes in program order, so the
    # drain still orders after everything.
    nc = self.nc
    probe = nc.sync.nop().ins
    wait_clock.add_sem_waits(probe, ScopedClock({None: tick_clock.global_clock}))
    waits = list(probe.sync_info.on_wait) if probe.sync_info else []
    probe.sync_info = mybir.SyncInfo(on_wait=waits[:1], on_update=[])
    for w in waits[1:]:
        extra = nc.sync.nop().ins
        extra.sync_info = mybir.SyncInfo(on_wait=[w], on_update=[])
    nc.sync.drain()
    nc.all_engine_barrier()
    assert self.sems is not None
    popped = nc._tile_sem_poison_stack.pop()
    assert popped is self._sem_poison
    nc.clear_and_free_semaphores(list(self.sems.allocated().values()))
    nc.all_engine_barrier()


tile.TileContext._drain_and_barrier = _patched_drain_and_barrier


def _split_excess_waits(nc: bass.Bass, max_waits: int = 1):
    # Same walrus limitation, general form: cap sync waits per instruction,
    # hoisting the excess onto same-engine NOPs inserted just before (the
    # engine executes its stream in order, so semantics are unchanged).
    for bb in nc.main_func.blocks:
        insts = list(bb.instructions)
        out, changed = [], False
        for ins in insts:
            si = ins.sync_info
            waits = list(si.on_wait) if (si is not None and si.on_wait) else []
            if len(waits) > max_waits:
                ups = list(si.on_update) if si.on_update else []
                for w in waits[:-max_waits]:
                    nop = mybir.InstNoOp(
                        name=nc.get_next_instruction_name(), ins=[], outs=[])
                    nop.engine = ins.engine
                    nop.sync_info = mybir.SyncInfo(on_wait=[w], on_update=[])
                    nc.register_instruction(nop)
                    out.append(nop)
                ins.sync_info = mybir.SyncInfo(
                    on_wait=waits[-max_waits:], on_update=ups)
                changed = True
            out.append(ins)
        if changed:
            bb.instructions = out


def build_nc(bench_iters: int = 0) -> bass.Bass:
    # bench_iters>0 wraps the compute body in a HW For_i loop so one
    # dispatch runs it N times (timing two N values cancels dispatch cost).
    nc = bass.Bass()
    x_dram = nc.declare_dram_parameter("pred_logit", [ROWS, C], bf16, isOutput=False)
    gt_dram = nc.declare_dram_parameter("gt", [ROWS, C], bf16, isOutput=False)
    out_dram = nc.declare_dram_parameter("partials", [N_SLICE, 512], f32, isOutput=True)
    ce_dram = nc.declare_dram_parameter("ce_part", [1, 512], f32, isOutput=True)

    from contextlib import ExitStack
    with tile.TileContext(nc) as tc, ExitStack() as es:
        consts = es.enter_context(tc.tile_pool(name="consts", bufs=1))
        xpool = es.enter_context(tc.tile_pool(name="xpool", bufs=2))
        tpool = es.enter_context(tc.tile_pool(name="tpool", bufs=2))
        rowp = es.enter_context(tc.tile_pool(name="rowp", bufs=2))
        ck = es.enter_context(tc.tile_pool(name="ck", bufs=2))
        psum = es.enter_context(tc.tile_pool(name="psum", bufs=1, space="PSUM"))

        ones = consts.tile([128, 1], bf16)
        nc.vector.memset(ones, 1.0)
        neg_ones = consts.tile([128, 1], bf16)
        nc.vector.memset(neg_ones, -1.0)
        eps_b = consts.tile([128, 1], f32)
        nc.vector.memset(eps_b, EPS * _EK_HALF)

        # Column-sum accumulators for contrib = Pa+Pb-Pc: one [1,512] row
        # per 512-column slice. PE output base partition must be 0/32/64,
        # so pack 3 slices per PSUM bank at those bases.
        banks = [psum.tile([128, 512], f32, name=f"csbank{i}", tag=f"csbank{i}")
                 for i in range((N_SLICE + 2) // 3)]
        def cs_ap(m):
            bank, base = banks[m // 3], 32 * (m % 3)
            return bank[base:base + 1, :]
        # CE accumulator: column sums of Pd from ALL slices superimposed
        # into one 512-wide row; the host sums the 512 values.
        ce_psum = psum.tile([1, 512], f32)

        N_XSUB = N_CHUNK   # one exp sub-chunk per chunk iteration
        XS = C // N_XSUB

        def alloc_block():
            xb = xpool.tile([128, C], bf16, tag="x")
            tb = tpool.tile([128, C], bf16, tag="t")
            s4 = rowp.tile([128, N_XSUB], f32, tag="s4")
            return xb, tb, s4

        def emit_exp_sub(b, tiles, ix):
            xb, tb, s4 = tiles
            r0 = b * 128
            xsl = slice(ix * XS, (ix + 1) * XS)
            nc.sync.dma_start(out=xb[:, xsl], in_=x_dram[r0:r0 + 128, xsl])
            nc.scalar.activation(
                out=tb[:, xsl], in_=xb[:, xsl], func=AF.Exp,
                accum_out=s4[:, ix:ix + 1],
            )

        def finish_block(tiles):
            xb, tb, s4 = tiles
            s = rowp.tile([128, 1], f32, tag="s")
            nc.vector.tensor_reduce(
                out=s[:], in_=s4[:], op=ALU.add, axis=mybir.AxisListType.X,
            )
            recip = rowp.tile([128, 1], f32, tag="recip")
            nc.vector.reciprocal(out=recip[:], in_=s[:])
            # lse - K, then negated in bf16 as a PE stationary vector: the
            # -(lse-K)*p matmul folds the softmax normalizer AND the K shift
            # into the column-sum accumulator (K*(g - u + p) = 0).
            lse = rowp.tile([128, 1], f32, tag="lse")
            nc.scalar.activation(out=lse[:], in_=s[:], func=AF.Ln, scale=_EMK)
            nlse16 = rowp.tile([128, 1], bf16, tag="nlse16")
            nc.vector.tensor_scalar_mul(out=nlse16[:], in0=lse[:], scalar1=-1.0)
            return xb, tb, recip, nlse16

        def emit_gt_lng(b, c):
            # gt DMA + lng run one chunk ahead of the rest of the pipeline
            r0, j0 = b * 128, c * F2
            gtc = ck.tile([128, F2], bf16, tag="gt")
            nc.sync.dma_start(out=gtc[:], in_=gt_dram[r0:r0 + 128, j0:j0 + F2])
            lng = ck.tile([128, F2], bf16, tag="lng")
            nc.scalar.activation(out=lng[:], in_=gtc[:], func=AF.Ln, scale=_EK)
            return gtc, lng

        # Fully interleaved schedule. Steady-state ACT stream per chunk is
        # [lng(next chunk), logm(this), exp-sub(next block)] ~ 5.7us, which
        # matches the DVE stream [xp, p, u, pb, pa, pc] ~ 5.7us; block
        # row-scalar work (reduce/recip/lse) rides in the seams.
        assert N_CHUNK == N_XSUB
        def emit_body():
            chunk_list = [(b, c) for b in range(N_BLK) for c in range(N_CHUNK)]
            t0 = alloc_block()
            for ix in range(N_XSUB):
                emit_exp_sub(0, t0, ix)
            state = {0: finish_block(t0)}
            pend = {}
            nxt = emit_gt_lng(0, 0)
            for idx, (b, c) in enumerate(chunk_list):
                j0 = c * F2
                gtc, lng = nxt
                xb, tb, recip, nlse16 = state[b]
                if idx + 1 < len(chunk_list):
                    nxt = emit_gt_lng(*chunk_list[idx + 1])

                # DVE: p, u first (no ACT deps this iteration)
                p = ck.tile([128, F2], bf16, tag="p")
                nc.vector.tensor_scalar(
                    out=p[:], in0=tb[:, j0:j0 + F2],
                    scalar1=recip[:], scalar2=None, op0=ALU.mult,
                )
                u = ck.tile([128, F2], bf16, tag="u")
                nc.vector.tensor_tensor(out=u[:], in0=gtc[:], in1=p[:], op=ALU.add)

                logm = ck.tile([128, F2], bf16, tag="logm")
                nc.scalar.activation(
                    out=logm[:], in_=u[:], func=AF.Ln, scale=_EK_HALF,
                    bias=eps_b[:],
                )

                # products; pb = p*x first (ready before lng/logm land)
                pb = ck.tile([128, F2], bf16, tag="pb", bufs=2)
                nc.vector.tensor_tensor(
                    out=pb[:], in0=p[:], in1=xb[:, j0:j0 + F2], op=ALU.mult)
                pa = ck.tile([128, F2], bf16, tag="pa", bufs=2)
                nc.vector.tensor_tensor(out=pa[:], in0=gtc[:], in1=lng[:], op=ALU.mult)
                pc = ck.tile([128, F2], bf16, tag="pc", bufs=2)
                nc.vector.tensor_tensor(out=pc[:], in0=u[:], in1=logm[:], op=ALU.mult)
                # CE plane g*x: raw inputs only, runs far ahead on GpSimd
                pd = ck.tile([128, F2], bf16, tag="pd", bufs=2)
                if PD_ON_GPSIMD:
                    nc.gpsimd.tensor_tensor(
                        out=pd[:], in0=gtc[:], in1=xb[:, j0:j0 + F2], op=ALU.mult)
                else:
                    nc.vector.tensor_tensor(
                        out=pd[:], in0=gtc[:], in1=xb[:, j0:j0 + F2], op=ALU.mult)

                # next block's exp pass, one sub-chunk per chunk iteration
                if b + 1 < N_BLK:
                    if c == 0:
                        pend[b + 1] = alloc_block()
                    emit_exp_sub(b + 1, pend[b + 1], c)
                    if c == N_CHUNK - 1:
                        state[b + 1] = finish_block(pend.pop(b + 1))

                for k in range(F2 // 512):
                    m = (j0 + k * 512) // 512
                    sl = slice(k * 512, (k + 1) * 512)
                    nc.tensor.matmul(
                        cs_ap(m), ones[:], pb[:, sl],
                        start=(b == 0), stop=False,
                    )
                    nc.tensor.matmul(
                        cs_ap(m), ones[:], pa[:, sl],
                        start=False, stop=False,
                    )
                    nc.tensor.matmul(
                        ce_psum[:], ones[:], pd[:, sl],
                        start=(b == 0 and m == 0), stop=False,
                    )
                    # -(lse-K)*colsum(p): normalizer + K-shift fold
                    nc.tensor.matmul(
                        cs_ap(m), nlse16[:], p[:, sl],
                        start=False, stop=False,
                    )
                    nc.tensor.matmul(
                        cs_ap(m), neg_ones[:], pc[:, sl],
                        start=False, stop=(b == N_BLK - 1),
                    )
                # CE normalizer: ce[0,0] -= sum_i (lse-K)_i once per block
                if c == N_CHUNK - 1:
                    nc.tensor.matmul(
                        ce_psum[0:1, 0:1], nlse16[:], ones[:],
                        start=False, stop=(b == N_BLK - 1),
                    )

        if bench_iters > 1:
            with tc.For_i(0, bench_iters, 1):
                emit_body()
        else:
            emit_body()

        # PSUM is not DMA-readable: bounce through SBUF via ScalarE.
        sb_banks = [consts.tile([128, 512], f32, name=f"sb_cs{i}", tag=f"sb_cs{i}")
                    for i in range(len(banks))]
        for i, bank in enumerate(banks):
            nc.scalar.copy(out=sb_banks[i][:], in_=bank[:])
        sb_ce = consts.tile([1, 512], f32)
        nc.scalar.copy(out=sb_ce[:], in_=ce_psum[:])
        for m in range(N_SLICE):
            bank, base = sb_banks[m // 3], 32 * (m % 3)
            nc.sync.dma_start(out=out_dram[m:m + 1, :], in_=bank[base:base + 1, :])
        nc.sync.dma_start(out=ce_dram[:], in_=sb_ce[:])

    _split_excess_waits(nc)
    return nc


_NC_CACHE = None
LAST_EXEC_NS = None
LAST_TRACE = None
LAST_PROFILE_JSON = None


def make_in_maps(pred_logit: np.ndarray, gt: np.ndarray) -> list[dict]:
    x16 = np.ascontiguousarray(pred_logit).astype(ml_dtypes.bfloat16)
    g16 = np.ascontiguousarray(gt).astype(ml_dtypes.bfloat16)
    return [
        {
            "pred_logit": x16[c * ROWS:(c + 1) * ROWS],
            "gt": g16[c * ROWS:(c + 1) * ROWS],
        }
        for c in range(N_CORES)
    ]


def kernel(pred_logit: np.ndarray, gt: np.ndarray) -> np.ndarray:
    global _NC_CACHE, LAST_EXEC_NS, LAST_TRACE, LAST_PROFILE_JSON
    if _NC_CACHE is None:
        _NC_CACHE = build_nc()
    nc = _NC_CACHE

    in_maps = make_in_maps(pred_logit, gt)
    res = run_bass_kernel_spmd(nc, in_maps, list(range(N_CORES)))
    if res.exec_time_ns is not None:
        LAST_EXEC_NS = res.exec_time_ns
        LAST_TRACE = res.instructions_and_trace
        LAST_PROFILE_JSON = res.profile_json

    w = (C - np.arange(C)).astype(np.float64)
    cjs_total = 0.0  # sum_ij w_j * contrib
    ce_total = 0.0   # sum_ij gt * logp
    for r in res.results:
        cs = r["partials"].astype(np.float64).reshape(C)
        cjs_total += np.dot(w, cs)
        # ce plane accumulated gt*(x - lse + K); remove the K shift
        # (rows of gt sum to 1, so sum_ij K*gt = K*ROWS per core)
        ce_total += float(r["ce_part"].astype(np.float64).sum()) - K_LSE * ROWS
    loss = -ce_total / B + 0.25 * cjs_total / B
    return np.array(loss, dtype=np.float32)


# revision 16
# speedup vs baseline: 2.5041x; 2.5041x over previous
"""CE + CJS loss kernel for Trainium2, data-parallel over 8 NeuronCores.

Math (reference):
    logp = log_softmax(pred_logit, axis=1)          # x - lse_i
    ce   = -mean_i( sum_j gt*logp )
    p    = softmax(pred_logit)
    m    = 0.5*(gt + p + EPS)
    contrib = gt*ln(gt) + p*logp - (gt+p)*ln(m)     # per element
    cjs  = 0.5 * sum_ij w_j * contrib_ij / B,  w_j = C - j
    loss = ce + 0.5*cjs

v2 design (vs the first working kernel: 241us HW):
  - Inputs are downcast to bf16 on the host: halves DMA (32MiB->16MiB
    per core) and puts every elementwise op in a fast mode.
  - 4-product form: contrib = g*lng + p*xp - u*logm with u = g+p.
    Column sums are PE matmuls with a ones vector; the minus sign is a
    neg_ones stationary vector, so no tensor-level subtracts are needed.
    With the same +K shift on lng, logm, and xp the shift cancels
    exactly: K*(g + p - u) = 0.
  - Engine balance per 2048-col chunk:
      ACT: lng=Ln(EK*g), logm=Ln(EK/2*u + eps'), (+exp pass amortized)
      DVE: xp, p (tensor_scalar 4x), u, Pb, Pa, Pc (tensor_tensor 2x)
      GpSimd: Pd = g*xp (CE plane) -- keeps it off the two hot engines
      PE : ones/neg_ones column-sum matmuls into PSUM
  - CE total = sum(Pd colsums) - K*ROWS per core (rows of gt sum to 1).
"""
import os

import numpy as np
import ml_dtypes

import concourse.bass as bass
import concourse.tile as tile
from concourse import mybir
from concourse.bass_utils import run_bass_kernel_spmd
from concourse.vector_clock import ScopedClock

B, C = 4096, 8192
N_CORES = 8
ROWS = B // N_CORES          # 512 rows per core
N_BLK = ROWS // 128          # 4 partition blocks
F2 = 2048                    # chunk width
N_CHUNK = C // F2            # chunks per block
N_SLICE = C // 512           # matmul column slices
EPS = 1e-8
# All log-magnitude tensors (lng, logm, x-lse) sit near -9.2 where bf16's
# ulp is 0.0625; shifting them by +K before the bf16 round shrinks the
# quantization bias ~8x. The same K on lng/logm/xp cancels exactly in
# g*lng + p*xp - u*logm; the CE plane carries it and the host removes it.
K_SHIFT = 9.2
_EK = float(np.float32(np.exp(K_SHIFT)))          # scale for Ln(gt)
_EK_HALF = float(np.float32(_EK) / np.float32(2.0))  # exact /2: same shift
_EMK = float(np.float32(np.exp(-K_SHIFT)))        # scale for lse
K_LSE = -float(np.log(np.float64(_EMK)))          # effective shift on x-lse

PD_ON_GPSIMD = False   # CE plane g*xp on GpSimd instead of DVE

f32 = mybir.dt.float32
bf16 = mybir.dt.bfloat16
AF = mybir.ActivationFunctionType
ALU = mybir.AluOpType


def _patched_drain_and_barrier(self, tick_clock, wait_clock):
    # Walrus CoreV3 codegen allows only ONE sync-wait command on a
    # Drain/NoOp (NO_STRUCT ctrl). The stock Tile tail drain carries one
    # wait per pending engine clock and fails to compile. Split the waits
    # across single-wait SP nops; SP executes in program order, so the
    # drain still orders after everything.
    nc = self.nc
    probe = nc.sync.nop().ins
    wait_clock.add_sem_waits(probe, ScopedClock({None: tick_clock.global_clock}))
    waits = list(probe.sync_info.on_wait) if probe.sync_info else []
    probe.sync_info = mybir.SyncInfo(on_wait=waits[:1], on_update=[])
    for w in waits[1:]:
        extra = nc.sync.nop().ins
        extra.sync_info = mybir.SyncInfo(on_wait=[w], on_update=[])
    nc.sync.drain()
    nc.all_engine_barrier()
    assert self.sems is not None
    popped = nc._tile_sem_poison_stack.pop()
    assert popped is self._sem_poison
    nc.clear_and_free_semaphores(list(self.sems.allocated().values()))
    nc.all_engine_barrier()


tile.TileContext._drain_and_barrier = _patched_drain_and_barrier


def _split_excess_waits(nc: bass.Bass, max_waits: int = 1):
    # Same walrus limitation, general form: cap sync waits per instruction,
    # hoisting the excess onto same-engine NOPs inserted just before (the
    # engine executes its stream in order, so semantics are unchanged).
    for bb in nc.main_func.blocks:
        insts = list(bb.instructions)
        out, changed = [], False
        for ins in insts:
            si = ins.sync_info
            waits = list(si.on_wait) if (si is not None and si.on_wait) else []
            if len(waits) > max_waits:
                ups = list(si.on_update) if si.on_update else []
                for w in waits[:-max_waits]:
                    nop = mybir.InstNoOp(
                        name=nc.get_next_instruction_name(), ins=[], outs=[])
                    nop.engine = ins.engine
                    nop.sync_info = mybir.SyncInfo(on_wait=[w], on_update=[])
                    nc.register_instruction(nop)
                    out.append(nop)
                ins.sync_info = mybir.SyncInfo(
                    on_wait=waits[-max_waits:], on_update=ups)
                changed = True
            out.append(ins)
        if changed:
            bb.instructions = out


def build_nc(bench_iters: int = 0) -> bass.Bass:
    # bench_iters>0 wraps the compute body in a HW For_i loop so one
    # dispatch runs it N times (timing two N values cancels dispatch cost).
    nc = bass.Bass()
    x_dram = nc.declare_dram_parameter("pred_logit", [ROWS, C], bf16, isOutput=False)
    gt_dram = nc.declare_dram_parameter("gt", [ROWS, C], bf16, isOutput=False)
    out_dram = nc.declare_dram_parameter("partials", [N_SLICE, 512], f32, isOutput=True)
    ce_dram = nc.declare_dram_parameter("ce_part", [1, 512], f32, isOutput=True)

    from contextlib import ExitStack
    with tile.TileContext(nc) as tc, ExitStack() as es:
        consts = es.enter_context(tc.tile_pool(name="consts", bufs=1))
        xpool = es.enter_context(tc.tile_pool(name="xpool", bufs=2))
        tpool = es.enter_context(tc.tile_pool(name="tpool", bufs=2))
        rowp = es.enter_context(tc.tile_pool(name="rowp", bufs=2))
        ck = es.enter_context(tc.tile_pool(name="ck", bufs=2))
        psum = es.enter_context(tc.tile_pool(name="psum", bufs=1, space="PSUM"))

        ones = consts.tile([128, 1], bf16)
        nc.vector.memset(ones, 1.0)
        neg_ones = consts.tile([128, 1], bf16)
        nc.vector.memset(neg_ones, -1.0)
        eps_b = consts.tile([128, 1], f32)
        nc.vector.memset(eps_b, EPS * _EK_HALF)

        # Column-sum accumulators for contrib = Pa+Pb-Pc: one [1,512] row
        # per 512-column slice. PE output base partition must be 0/32/64,
        # so pack 3 slices per PSUM bank at those bases.
        banks = [psum.tile([128, 512], f32, name=f"csbank{i}", tag=f"csbank{i}")
                 for i in range((N_SLICE + 2) // 3)]
        def cs_ap(m):
            bank, base = banks[m // 3], 32 * (m % 3)
            return bank[base:base + 1, :]
        # CE accumulator: column sums of Pd from ALL slices superimposed
        # into one 512-wide row; the host sums the 512 values.
        ce_psum = psum.tile([1, 512], f32)

        N_XSUB = N_CHUNK   # one exp sub-chunk per chunk iteration
        XS = C // N_XSUB

        def alloc_block():
            xb = xpool.tile([128, C], bf16, tag="x")
            tb = tpool.tile([128, C], bf16, tag="t")
            s4 = rowp.tile([128, N_XSUB], f32, tag="s4")
            return xb, tb, s4

        def emit_exp_sub(b, tiles, ix):
            xb, tb, s4 = tiles
            r0 = b * 128
            xsl = slice(ix * XS, (ix + 1) * XS)
            nc.sync.dma_start(out=xb[:, xsl], in_=x_dram[r0:r0 + 128, xsl])
            nc.scalar.activation(
                out=tb[:, xsl], in_=xb[:, xsl], func=AF.Exp,
                accum_out=s4[:, ix:ix + 1],
            )

        def finish_block(tiles):
            xb, tb, s4 = tiles
            s = rowp.tile([128, 1], f32, tag="s")
            nc.vector.tensor_reduce(
                out=s[:], in_=s4[:], op=ALU.add, axis=mybir.AxisListType.X,
            )
            recip = rowp.tile([128, 1], f32, tag="recip")
            nc.vector.reciprocal(out=recip[:], in_=s[:])
            # lse - K, then negated in bf16 as a PE stationary vector: the
            # -(lse-K)*p matmul folds the softmax normalizer AND the K shift
            # into the column-sum accumulator (K*(g - u + p) = 0).
            lse = rowp.tile([128, 1], f32, tag="lse")
            nc.scalar.activation(out=lse[:], in_=s[:], func=AF.Ln, scale=_EMK)
            nlse16 = rowp.tile([128, 1], bf16, tag="nlse16")
            nc.vector.tensor_scalar_mul(out=nlse16[:], in0=lse[:], scalar1=-1.0)
            return xb, tb, recip, nlse16

        def emit_gt_lng(b, c):
            # gt DMA + lng run one chunk ahead of the rest of the pipeline
            r0, j0 = b * 128, c * F2
            gtc = ck.tile([128, F2], bf16, tag="gt")
            nc.sync.dma_start(out=gtc[:], in_=gt_dram[r0:r0 + 128, j0:j0 + F2])
            lng = ck.tile([128, F2], bf16, tag="lng")
            nc.scalar.activation(out=lng[:], in_=gtc[:], func=AF.Ln, scale=_EK)
            return gtc, lng

        # Fully interleaved schedule. Steady-state ACT stream per chunk is
        # [lng(next chunk), logm(this), exp-sub(next block)] ~ 5.7us, which
        # matches the DVE stream [xp, p, u, pb, pa, pc] ~ 5.7us; block
        # row-scalar work (reduce/recip/lse) rides in the seams.
        assert N_CHUNK == N_XSUB
        def emit_body():
            chunk_list = [(b, c) for b in range(N_BLK) for c in range(N_CHUNK)]
            t0 = alloc_block()
            for ix in range(N_XSUB):
                emit_exp_sub(0, t0, ix)
            state = {0: finish_block(t0)}
            pend = {}
            nxt = emit_gt_lng(0, 0)
            for idx, (b, c) in enumerate(chunk_list):
                j0 = c * F2
                gtc, lng = nxt
                xb, tb, recip, nlse16 = state[b]
                if idx + 1 < len(chunk_list):
                    nxt = emit_gt_lng(*chunk_list[idx + 1])

                # DVE: p, u first (no ACT deps this iteration)
                p = ck.tile([128, F2], bf16, tag="p")
                nc.vector.tensor_scalar(
                    out=p[:], in0=tb[:, j0:j0 + F2],
                    scalar1=recip[:], scalar2=None, op0=ALU.mult,
                )
                u = ck.tile([128, F2], bf16, tag="u")
                nc.vector.tensor_tensor(out=u[:], in0=gtc[:], in1=p[:], op=ALU.add)

                logm = ck.tile([128, F2], bf16, tag="logm")
                nc.scalar.activation(
                    out=logm[:], in_=u[:], func=AF.Ln, scale=_EK_HALF,
                    bias=eps_b[:],
                )

                # products; pb = p*x first (ready before lng/logm land)
                pb = ck.tile([128, F2], bf16, tag="pb", bufs=2)
                nc.vector.tensor_tensor(
                    out=pb[:], in0=p[:], in1=xb[:, j0:j0 + F2], op=ALU.mult)
                pa = ck.tile([128, F2], bf16, tag="pa", bufs=2)
                nc.vector.tensor_tensor(out=pa[:], in0=gtc[:], in1=lng[:], op=ALU.mult)
                pc = ck.tile([128, F2], bf16, tag="pc", bufs=2)
                nc.vector.tensor_tensor(out=pc[:], in0=u[:], in1=logm[:], op=ALU.mult)
                # CE plane g*x: raw inputs only, runs far ahead on GpSimd
                pd = ck.tile([128, F2], bf16, tag="pd", bufs=2)
                if PD_ON_GPSIMD:
                    nc.gpsimd.tensor_tensor(
                        out=pd[:], in0=gtc[:], in1=xb[:, j0:j0 + F2], op=ALU.mult)
                else:
                    nc.vector.tensor_tensor(
                        out=pd[:], in0=gtc[:], in1=xb[:, j0:j0 + F2], op=ALU.mult)

                # next block's exp pass, one sub-chunk per chunk iteration
                if b + 1 < N_BLK:
                    if c == 0:
                        pend[b + 1] = alloc_block()
                    emit_exp_sub(b + 1, pend[b + 1], c)
                    if c == N_CHUNK - 1:
                        state[b + 1] = finish_block(pend.pop(b + 1))

                for k in range(F2 // 512):
                    m = (j0 + k * 512) // 512
                    sl = slice(k * 512, (k + 1) * 512)
                    nc.tensor.matmul(
                        cs_ap(m), ones[:], pb[:, sl],
                        start=(b == 0), stop=False,
                    )
                    nc.tensor.matmul(
                        cs_ap(m), ones[:], pa[:, sl],
                        start=False, stop=False,
                    )
                    nc.tensor.matmul(
                        ce_psum[:], ones[:], pd[:, sl],
                        start=(b == 0 and m == 0), stop=False,
                    )
                    # -(lse-K)*colsum(p): normalizer + K-shift fold
                    nc.tensor.matmul(
                        cs_ap(m), nlse16[:], p[:, sl],
                        start=False, stop=False,
                    )
                    nc.tensor.matmul(
                        cs_ap(m), neg_ones[:], pc[:, sl],
                        start=False, stop=(b == N_BLK - 1),
                    )
                # CE normalizer: ce[0,0] -= sum_i (lse-K)_i once per block
                if c == N_CHUNK - 1:
                    nc.tensor.matmul(
                        ce_psum[0:1, 0:1], nlse16[:], ones[:],
                        start=False, stop=(b == N_BLK - 1),
                    )

        if bench_iters > 1:
            with tc.For_i(0, bench_iters, 1):
                emit_body()
        else:
            emit_body()

        # PSUM is not DMA-readable: bounce through SBUF via ScalarE.
        sb_banks = [consts.tile([128, 512], f32, name=f"sb_cs{i}", tag=f"sb_cs{i}")
                    for i in range(len(banks))]
        for i, bank in enumerate(banks):
            nc.scalar.copy(out=sb_banks[i][:], in_=bank[:])
        sb_ce = consts.tile([1, 512], f32)
        nc.scalar.copy(out=sb_ce[:], in_=ce_psum[:])
        for m in range(N_SLICE):
            bank, base = sb_banks[m // 3], 32 * (m % 3)
            nc.sync.dma_start(out=out_dram[m:m + 1, :], in_=bank[base:base + 1, :])
        nc.sync.dma_start(out=ce_dram[:], in_=sb_ce[:])

    _split_excess_waits(nc)
    return nc


_NC_CACHE = None
LAST_EXEC_NS = None
LAST_TRACE = None
LAST_PROFILE_JSON = None


def make_in_maps(pred_logit: np.ndarray, gt: np.ndarray) -> list[dict]:
    x16 = np.ascontiguousarray(pred_logit).astype(ml_dtypes.bfloat16)
    g16 = np.ascontiguousarray(gt).astype(ml_dtypes.bfloat16)
    return [
        {
            "pred_logit": x16[c * ROWS:(c + 1) * ROWS],
            "gt": g16[c * ROWS:(c + 1) * ROWS],
        }
        for c in range(N_CORES)
    ]


def kernel(pred_logit: np.ndarray, gt: np.ndarray) -> np.ndarray:
    global _NC_CACHE, LAST_EXEC_NS, LAST_TRACE, LAST_PROFILE_JSON
    if _NC_CACHE is None:
        _NC_CACHE = build_nc()
    nc = _NC_CACHE

    in_maps = make_in_maps(pred_logit, gt)
    res = run_bass_kernel_spmd(nc, in_maps, list(range(N_CORES)))
    if res.exec_time_ns is not None:
        LAST_EXEC_NS = res.exec_time_ns
        LAST_TRACE = res.instructions_and_trace
        LAST_PROFILE_JSON = res.profile_json

    w = (C - np.arange(C)).astype(np.float64)
    cjs_total = 0.0  # sum_ij w_j * contrib
    ce_total = 0.0   # sum_ij gt * logp
    for r in res.results:
        cs = r["partials"].astype(np.float64).reshape(C)
        cjs_total += np.dot(w, cs)
        # ce plane accumulated gt*(x - lse + K); remove the K shift
        # (rows of gt sum to 1, so sum_ij K*gt = K*ROWS per core)
        ce_total += float(r["ce_part"].astype(np.float64).sum()) - K_LSE * ROWS
    loss = -ce_total / B + 0.25 * cjs_total / B
    return np.array(loss, dtype=np.float32)
